# revision 67
# baseline (speedup 1.0000x reference)
"""Multi-head attention (B=2, N=2048, C=1024, H=16, D=64) on 8 TRN2 cores.

Sharding: tensor-parallel over heads — 2 heads per core. Each core computes
Q/K/V projections for its 2 heads, attention, and a partial output
projection (its heads' slice of Wo). Host sums the 8 partial outputs + bo.

Per-core dataflow (all matmul inputs bf16, PSUM accumulation fp32):
  xT [1024, 4096] (x transposed on host, replicated to all cores),
  loaded token-major as 8x8 tiles [128, 512] so the first projection
  chain can start after ~1MB instead of 4MB.
  QT/KT = W.T @ x.T   -> [128 (2 heads x 64), 4096]  (lhsT=W chunk, rhs=xT)
  VT likewise, then PE-transposed into v_aug [keys, 65] per head
  (65th column = ones -> softmax denominator comes out of the ctx matmul)
  S^T = K @ Q.T  -> [keys, q] in PSUM; the two heads' matmuls run
  concurrently on the PE row-halves (row_grp tiling). exp on ScalarE.
  ctx^T_aug [65, q] = v_aug.T @ expS^T  (row 64 = denominator)
  normalize: recip(row 64), gpsimd partition_broadcast, DVE multiply
  out_partial [4096, 1024] = ctx^T.T @ Wo_slice, copied to bf16 and
  summed on host.

Scheduling: projection chains for K/V beyond chunk 0 are woven into the
attention kc-loops as PE filler (region-level Tile deps allow partial
reads of KTt/vaug), and each chunk's output projection is woven into the
next chunk's kc-loop so the single po PSUM bank never stalls the PE.
The normalize chain is kept to few, fp32 engine ops (engine ops carry
~650ns fixed cost, and the woven units gate on counting semaphores =
all earlier DVE/GpSimd work), the last kc's exp is split in half to
give the chunk tail a head start, and the final chunk's tail pipelines
its out-proj matmuls over the freed psS banks with PSUM->bf16 casts
alternating ScalarE/DVE.
The 1/sqrt(D) scale is folded into Wq/bq on the host (exact: 0.125).

Measured (8 axon trn2 cores, nominal 2.4GHz clock): ~222.5us HW exec
(222345/222681), rel err 2.4e-3 (baseline: 250575ns). Note the device clock is bimodal
across runs (~379ns vs ~454ns per 512-col matmul = 2.4 vs 2.0 GHz) and
there is +-2us run noise at fixed clock; compare timings via the median
matmul duration in the NTFF trace.
"""

import numpy as np
import ml_dtypes

import concourse.bass as bass
from concourse import bacc
import concourse.tile as tile
from concourse import mybir, library_config
from concourse.bass_utils import run_bass_kernel_spmd

BF16 = mybir.dt.bfloat16
F32 = mybir.dt.float32

B, N, C = 2, 2048, 1024
H, D = 16, 64
T = B * N              # 4096 tokens
HPC = H // 8           # heads per core = 2
DPC = HPC * D          # head dims per core = 128


def build_core_program(nc):
    """Emit the per-core SPMD program. Same program on all 8 cores;
    per-core data differences come from the input maps."""
    xT = nc.dram_tensor("xT", [C, T], BF16, kind="ExternalInput").ap()
    wq = nc.dram_tensor("wq", [C, DPC], BF16, kind="ExternalInput").ap()
    wk = nc.dram_tensor("wk", [C, DPC], BF16, kind="ExternalInput").ap()
    wv = nc.dram_tensor("wv", [C, DPC], BF16, kind="ExternalInput").ap()
    wo = nc.dram_tensor("wo", [DPC, C], BF16, kind="ExternalInput").ap()
    bqkv = nc.dram_tensor("bqkv", [DPC, 3], F32, kind="ExternalInput").ap()
    iden = nc.dram_tensor("iden", [128, 128], BF16, kind="ExternalInput").ap()
    out = nc.dram_tensor("out", [T, C], BF16, kind="ExternalOutput").ap()

    KCH = C // 128     # 8 contraction chunks for projections
    NCH = T // 512     # 8 token chunks of 512
    KT16 = N // 128    # 16 key tiles per batch

    with tile.TileContext(nc) as tc:
        with tc.tile_pool(name="singles", bufs=1) as singles:
            nc.gpsimd.load_library(library_config.proxy)

            # DMA priority order: iden (warmup), wv weights + x chunk 0
            # (first projection chain), then the rest; wo last (first used
            # ~40us in).
            id_sb = singles.tile([128, 128], BF16, tag="iden")
            nc.sync.dma_start(out=id_sb, in_=iden)

            w_sb = {}
            w_tiles = {}
            for nm, w in (("wv", wv), ("wk", wk), ("wq", wq)):
                t = singles.tile([128, KCH, DPC], BF16, tag=f"w{nm}",
                                 name=f"w{nm}")
                w_tiles[nm] = (t, w)
                w_sb[nm] = [t[:, k, :] for k in range(KCH)]

            def dma_w(nm):
                t, w = w_tiles[nm]
                nc.sync.dma_start(
                    out=t, in_=w.rearrange("(k p) j -> p k j", p=128))

            # per-(k, token-chunk) [128, 512] tiles, DMA'd separately so a
            # projection chain's k-th matmul can start as soon as its own
            # 128KB tile lands (progressive arrival beats fewer/bigger
            # transfers here — the source rows are 1KB either way)
            xt = [[singles.tile([128, 512], BF16, tag=f"xt{k}_{t}",
                                name=f"xt{k}_{t}")
                   for t in range(NCH)] for k in range(KCH)]

            def dma_x(t):
                for k in range(KCH):
                    nc.sync.dma_start(
                        out=xt[k][t],
                        in_=xT[k * 128:(k + 1) * 128, t * 512:(t + 1) * 512])

            bqkv_sb = singles.tile([DPC, 3], F32, tag="bqkv")
            wo_sb = singles.tile([DPC, C], BF16, tag="wo")

            dma_w("wv")
            dma_x(0)
            dma_w("wk")
            dma_x(1)
            dma_w("wq")
            nc.sync.dma_start(out=bqkv_sb, in_=bqkv)
            for t in range(2, NCH):
                dma_x(t)
            nc.sync.dma_start(out=wo_sb, in_=wo)

            b_sb = {"bq": bqkv_sb[:, 0:1], "bk": bqkv_sb[:, 1:2],
                    "bv": bqkv_sb[:, 2:3]}

            # warmup source: memset early so the warmup matmuls (inside the
            # PSUM pools below) don't queue behind the vaug memsets on DVE
            wsrc = singles.tile([128, 128], BF16, tag="wsrc")
            nc.vector.memset(wsrc, 0.5)

            QT = singles.tile([128, T], BF16, tag="QT")
            KTt = singles.tile([128, T], BF16, tag="KT")
            VT = singles.tile([128, T], BF16, tag="VT")
            ctxTn = singles.tile([128, T], BF16, tag="ctxTn")
            vaug = [[singles.tile([128, KT16, D + 1], BF16,
                                  tag=f"vaug{b}{h}", name=f"vaug{b}{h}")
                     for h in range(HPC)] for b in range(B)]
            for b in range(B):
                for h in range(HPC):
                    nc.vector.memset(vaug[b][h], 1.0)

            # One unified PSUM layout for the whole kernel (8 banks:
            # pj 1 + po 1 + s 2x2 + ctx 2).
            with tc.tile_pool(name="psP", bufs=1, space="PSUM") as psP, \
                    tc.tile_pool(name="psO", bufs=1, space="PSUM") as psO, \
                    tc.tile_pool(name="psS", bufs=2, space="PSUM") as psS, \
                    tc.tile_pool(name="psC", bufs=1, space="PSUM") as psC, \
                    tc.tile_pool(name="esb", bufs=6) as esb, \
                    tc.tile_pool(name="nrm", bufs=3) as nrm, \
                    tc.tile_pool(name="csb", bufs=3) as csb, \
                    tc.tile_pool(name="osb", bufs=3) as osb:

                # keep PE busy (p-state ramp) while xT streams in; matmul
                # on a memset tile so warmup needs no DMA (starts ~4.5us
                # in, right after the engine preamble)
                for wu in range(20):
                    ptw = psS.tile([128, 128], F32, tag="s", name="ptw")
                    nc.tensor.matmul(out=ptw, lhsT=wsrc, rhs=wsrc,
                                     start=True, stop=True)
                startup_ctr = [0]

                def emit_proj(nm, dstT, nch, act_bias=False):
                    # Startup chains rotate over 4 banks (po + the ctx pair
                    # are idle pre-attention) with the bias on ScalarE;
                    # mid-attention fill chains use the single pj bank with
                    # the bias on DVE (chains are spaced kc's apart so the
                    # drain hides).
                    if act_bias:
                        slots = [(psO, "po"), (psC, "ctx0"), (psC, "ctx1"),
                                 (psP, "pj")]
                        pool, tg = slots[startup_ctr[0] % len(slots)]
                        startup_ctr[0] += 1
                    else:
                        pool, tg = psP, "pj"
                    ps = pool.tile([128, 512], F32, tag=tg, name="pj")
                    for k in range(KCH):
                        nc.tensor.matmul(
                            out=ps, lhsT=w_sb[nm][k], rhs=xt[k][nch],
                            start=(k == 0), stop=(k == KCH - 1))
                    dst = dstT[:, nch * 512:(nch + 1) * 512]
                    if act_bias:
                        # ScalarE is idle before attention starts
                        nc.scalar.activation(
                            out=dst, in_=ps,
                            func=mybir.ActivationFunctionType.Identity,
                            bias=b_sb["b" + nm[1]], scale=1.0)
                    else:
                        nc.vector.tensor_scalar_add(
                            out=dst, in0=ps, scalar1=b_sb["b" + nm[1]])
                    if nm == "wv":
                        # transpose the 4 just-projected 128-token tiles of V
                        # into v_aug [keys, 65] per head
                        for t16 in range(nch * 4, nch * 4 + 4):
                            b, bt = divmod(t16, KT16)
                            pt = psO.tile([128, 128], BF16, tag="po",
                                          name="pt")
                            base = t16 * 128
                            nc.tensor.transpose(
                                pt, VT[:, base:base + 128], id_sb)
                            nc.vector.tensor_copy(
                                out=vaug[b][0][:, bt, 0:D], in_=pt[:, 0:D])
                            nc.vector.tensor_copy(
                                out=vaug[b][1][:, bt, 0:D], in_=pt[:, D:2 * D])

                def emit_attention(b, qch, fillers, last=False,
                                   alt_banks=False, units_in_psC=False):
                    q0 = b * N + qch * 512
                    if alt_banks:
                        # chunks 5-7 have no projection chains left, so the
                        # pj/po banks are free: park the ctx accumulators
                        # there and let this chunk's woven out-proj units
                        # use psC — decouples the chunk boundary from the
                        # previous normalize's PSUM reads.
                        ctx = [psP.tile([D + 1, 512], F32, tag="pj",
                                        name="ctx0"),
                               psO.tile([D + 1, 512], F32, tag="po",
                                        name="ctx1")]
                    else:
                        ctx = [psC.tile([D + 1, 512], F32, tag=f"ctx{h}",
                                        name=f"ctx{h}")
                               for h in range(HPC)]

                    # 2-kc groups: [S,S][exp,exp][fills][C,C]. Entering or
                    # leaving a row-tiled scores pair costs ~+100ns on the
                    # PE (the full-array successor waits for both subarray
                    # pipelines); grouping halves those crossings and the
                    # fills sit on the S->C boundary that was paying the
                    # penalty anyway.
                    for kc0 in range(0, KT16, 2):
                        pSs = []
                        for kc in (kc0, kc0 + 1):
                            k0 = b * N + kc * 128
                            pS = psS.tile([128, 1024], F32, tag="s",
                                          name="s")
                            pSs.append(pS)
                            for h in range(HPC):
                                nc.tensor.matmul(
                                    out=pS[:, h * 512:(h + 1) * 512],
                                    lhsT=KTt[h * D:(h + 1) * D,
                                             k0:k0 + 128],
                                    rhs=QT[h * D:(h + 1) * D,
                                           q0:q0 + 512],
                                    start=True, stop=True)
                        eSs = []
                        for kc, pS in zip((kc0, kc0 + 1), pSs):
                            eS = esb.tile([128, 1024], BF16, tag="e",
                                          name="e")
                            eSs.append(eS)
                            if kc == KT16 - 1:
                                # split the last exp so ctx(15) and the
                                # normalize get a ~0.5us head start
                                for h in range(HPC):
                                    nc.scalar.activation(
                                        eS[:, h * 512:(h + 1) * 512],
                                        pS[:, h * 512:(h + 1) * 512],
                                        mybir.ActivationFunctionType.Exp)
                            else:
                                nc.scalar.activation(
                                    eS, pS,
                                    mybir.ActivationFunctionType.Exp)
                        for kc in (kc0, kc0 + 1):
                            for f in fillers.get(kc, ()):
                                f()
                        for kc, eS in zip((kc0, kc0 + 1), eSs):
                            for h in range(HPC):
                                nc.tensor.matmul(
                                    out=ctx[h],
                                    lhsT=vaug[b][h][:, kc, :],
                                    rhs=eS[:, h * 512:(h + 1) * 512],
                                    start=(kc == 0),
                                    stop=(kc == KT16 - 1))
                    # normalize: rows 0..63 / row 64, into stacked ctxTn.
                    # The woven out-proj units wait on COUNTING semaphores —
                    # all earlier DVE/GpSimd work must finish — so minimize
                    # the op count: ONE [65,512] f32 copy per head (frees
                    # the PSUM bank and feeds both the reciprocal and the
                    # multiplies), reciprocal off its SBUF row 64, f32
                    # broadcast (same ~1us as bf16, no cast needed). In the
                    # last chunk the copies go to ScalarE (idle once the
                    # exps are done) to shorten the DVE chain further.
                    rcs = []
                    ctxss = []
                    for h in range(HPC):
                        cp = csb.tile([D + 1, 512], F32, tag=f"ctxs{h}",
                                      name=f"ctxs{h}")
                        if last:
                            nc.scalar.copy(cp, ctx[h])
                        else:
                            nc.vector.tensor_copy(cp, ctx[h])
                        # the custom-DVE reciprocal needs a partition-0 fp32
                        # SBUF input — a base-64 slice (or PSUM) gives
                        # garbage, so extract the denominator row first
                        dn = nrm.tile([1, 512], F32, tag=f"dn{h}",
                                      name=f"dn{h}")
                        nc.vector.tensor_copy(dn, ctx[h][D:D + 1, :])
                        rc = nrm.tile([1, 512], F32, tag=f"rc{h}",
                                      name=f"rc{h}")
                        nc.vector.reciprocal_approx_fast(rc, dn)
                        ctxss.append(cp)
                        rcs.append(rc)
                    bcs = []
                    for h in range(HPC):
                        bc = nrm.tile([D, 512], F32, tag=f"bc{h}",
                                      name=f"bc{h}")
                        nc.gpsimd.partition_broadcast(bc, rcs[h])
                        bcs.append(bc)

                    def emit_mul(h, lo, hi):
                        # h0's multiply runs on GpSimd (idle engine, no
                        # partition shift); h1's needs the base-64 write,
                        # keep it on DVE where that is proven.
                        eng = nc.gpsimd if h == 0 else nc.vector
                        eng.tensor_mul(
                            out=ctxTn[h * D:(h + 1) * D, q0 + lo:q0 + hi],
                            in0=ctxss[h][0:D, lo:hi], in1=bcs[h][:, lo:hi])

                    def emit_op(t4, nch2, po, ot_slice):
                        tok = q0 + t4 * 128
                        nc.tensor.matmul(
                            out=po,
                            lhsT=ctxTn[:, tok:tok + 128],
                            rhs=wo_sb[:, nch2 * 512:(nch2 + 1) * 512],
                            start=True, stop=True)
                        nc.vector.tensor_copy(ot_slice, po)

                    if not last:
                        for h in range(HPC):
                            emit_mul(h, 0, 512)

                        # out-projection as 8 weavable units; each does one
                        # [128-token x 512-col] matmul + bf16 copy, DMA per
                        # 4-tile group. Woven into the NEXT chunk's kc loop
                        # so the po bank drain hides under attention.
                        ot_box = [None]

                        def op_unit(j):
                            nch2, t4 = divmod(j, 4)

                            def run():
                                if t4 == 0:
                                    ot_box[0] = osb.tile(
                                        [128, 4, 512], BF16, tag="ot",
                                        name="ot")
                                if units_in_psC:
                                    po = psC.tile([128, 512], F32,
                                                  tag=f"ctx{j % 2}",
                                                  name="po")
                                else:
                                    po = psO.tile([128, 512], F32,
                                                  tag="po", name="po")
                                emit_op(t4, nch2, po, ot_box[0][:, t4, :])
                                if t4 == 3:
                                    nc.sync.dma_start(
                                        out=out[q0:q0 + 512,
                                                nch2 * 512:(nch2 + 1) * 512
                                                ].rearrange(
                                                    "(t p) c -> p t c", p=128),
                                        in_=ot_box[0])
                            return run

                        return [op_unit(j) for j in range(8)]

                    # final chunk: per-qtile pipeline, matmuls on the four
                    # freed psS banks, PSUM->bf16 casts alternating between
                    # ScalarE (idle after the last exp) and DVE so they
                    # drain in parallel with the matmul stream
                    for t4 in range(4):
                        for h in range(HPC):
                            emit_mul(h, t4 * 128, (t4 + 1) * 128)
                    for t4 in range(4):
                        for nch2 in range(2):
                            tok = q0 + t4 * 128
                            po = psS.tile([128, 512], F32, tag="s",
                                          name="po")
                            ot = osb.tile([128, 512], BF16, tag="otl",
                                          name="otl", bufs=4)
                            nc.tensor.matmul(
                                out=po,
                                lhsT=ctxTn[:, tok:tok + 128],
                                rhs=wo_sb[:, nch2 * 512:(nch2 + 1) * 512],
                                start=True, stop=True)
                            if nch2 == 0:
                                nc.scalar.copy(ot, po)
                            else:
                                nc.vector.tensor_copy(ot, po)
                            nc.sync.dma_start(
                                out=out[tok:tok + 128,
                                        nch2 * 512:(nch2 + 1) * 512],
                                in_=ot)
                    return None

                # Startup: only chunk-0 projections of V/K/Q, then attention
                # begins; the rest of b0's K/V chains weave into chunk 0's
                # kc loop (scores kc needs K chunk kc//4, ctx kc needs V
                # chunk kc//4, each emitted >=2 kc ahead).
                emit_proj("wv", VT, 0, act_bias=True)
                emit_proj("wk", KTt, 0, act_bias=True)
                emit_proj("wq", QT, 0, act_bias=True)

                # proj fill chains per chunk index i (woven at odd kc slots;
                # out-proj units go at even slots). All Q projections are
                # front-loaded into chunks 0-4 so chunks 5-7 leave pj/po
                # free for their ctx accumulators (alt_banks).
                proj_fills = {
                    0: {1: [("wk", KTt, 1)], 3: [("wv", VT, 1)],
                        5: [("wk", KTt, 2)], 7: [("wv", VT, 2)],
                        9: [("wk", KTt, 3)], 11: [("wv", VT, 3)]},
                    1: {5: [("wk", KTt, 4)], 7: [("wv", VT, 4)]},
                    2: {5: [("wk", KTt, 5)], 7: [("wv", VT, 5)]},
                    3: {5: [("wk", KTt, 6)], 7: [("wv", VT, 6)]},
                    4: {1: [("wk", KTt, 7)], 3: [("wv", VT, 7)]},
                }
                pending_ops = None
                for i in range(8):
                    fillers = {}
                    for kc, chains in proj_fills.get(i, {}).items():
                        fillers[kc] = [
                            (lambda a: lambda: emit_proj(*a))(a)
                            for a in chains]
                    if pending_ops is not None:
                        # weave previous chunk's out-proj into the kc loop.
                        # Unit 0 reads ctxTn, ready only ~4-5us after the
                        # boundary; chunks with proj fill chains (i<=4)
                        # have enough other PE work for early slots, the
                        # later chunks need the weave pushed to slot 6+.
                        if i <= 4:
                            slots = [2, 4, 6, 8, 10, 12, 14, 15]
                        else:
                            slots = [6, 7, 8, 9, 10, 11, 12, 13]
                        for j, u in enumerate(pending_ops):
                            fillers.setdefault(slots[j], []).append(u)
                    if i + 1 < 8:
                        # slot 9 (not 13): the Q chain's DVE bias must
                        # drain before the boundary normalize chain, which
                        # gates the next chunk's ctx banks via counting
                        # semaphores. Chunk 0's slots 9/11 hold wk3/wv3,
                        # so its Q goes at 13 (its boundary is clean
                        # anyway — no unit weave in chunk 1's early slots).
                        qslot = 13 if i == 0 else 9
                        fillers.setdefault(qslot, []).append(
                            (lambda n: lambda: emit_proj("wq", QT, n))(i + 1))
                    pending_ops = emit_attention(
                        i // 4, i % 4, fillers, last=(i == 7))
    return nc


_NC_CACHE = None


def _get_nc():
    global _NC_CACHE
    if _NC_CACHE is None:
        nc = bacc.Bacc("TRN2", target_bir_lowering=False)
        build_core_program(nc)
        nc.finalize()
        _NC_CACHE = nc
    return _NC_CACHE


def make_in_maps(x, Wq, bq, Wk, bk, Wv, bv, Wo):
    bf = ml_dtypes.bfloat16
    x = np.asarray(x, np.float32).reshape(T, C)
    xT_bf = np.ascontiguousarray(x.T).astype(bf)
    iden = np.eye(128, dtype=bf)
    Wq = np.asarray(Wq, np.float32)
    Wk = np.asarray(Wk, np.float32)
    Wv = np.asarray(Wv, np.float32)
    Wo = np.asarray(Wo, np.float32)
    bq = np.asarray(bq, np.float32)
    bk = np.asarray(bk, np.float32)
    bv = np.asarray(bv, np.float32)
    in_maps = []
    for cidx in range(8):
        hs = slice(cidx * DPC, (cidx + 1) * DPC)
        in_maps.append(dict(
            xT=xT_bf,
            wq=np.ascontiguousarray(Wq[:, hs] * 0.125).astype(bf),
            wk=np.ascontiguousarray(Wk[:, hs]).astype(bf),
            wv=np.ascontiguousarray(Wv[:, hs]).astype(bf),
            wo=np.ascontiguousarray(Wo[hs, :]).astype(bf),
            bqkv=np.stack([bq[hs] * 0.125, bk[hs], bv[hs]],
                          axis=1).astype(np.float32),
            iden=iden,
        ))
    return in_maps


def kernel(x, Wq, bq, Wk, bk, Wv, bv, Wo, bo, _trace=False, _trace_kwargs=None):
    in_maps = make_in_maps(x, Wq, bq, Wk, bk, Wv, bv, Wo)
    nc = _get_nc()
    res = run_bass_kernel_spmd(
        nc, in_maps, core_ids=list(range(8)),
        trace=_trace, **(_trace_kwargs or {}))
    acc = res.results[0]["out"].astype(np.float32)
    for cidx in range(1, 8):
        acc += res.results[cidx]["out"]
    acc += np.asarray(bo, np.float32)[None, :]
    out = acc.reshape(B, N, C)
    kernel.last_results = res
    return out


# revision 68
# speedup vs baseline: 1.0108x; 1.0108x over previous
"""Multi-head attention (B=2, N=2048, C=1024, H=16, D=64) on 8 TRN2 cores.

Sharding: tensor-parallel over heads — 2 heads per core. Each core computes
Q/K/V projections for its 2 heads, attention, and a partial output
projection (its heads' slice of Wo). Host sums the 8 partial outputs + bo.

Per-core dataflow (all matmul inputs bf16, PSUM accumulation fp32):
  xT [1024, 4096] (x transposed on host, replicated to all cores),
  loaded token-major as 8x8 tiles [128, 512] so the first projection
  chain can start after ~1MB instead of 4MB.
  QT/KT = W.T @ x.T   -> [128 (2 heads x 64), 4096]  (lhsT=W chunk, rhs=xT)
  VT likewise, then PE-transposed into v_aug [keys, 65] per head
  (65th column = ones -> softmax denominator comes out of the ctx matmul)
  S^T = K @ Q.T  -> [keys, q] in PSUM; the two heads' matmuls run
  concurrently on the PE row-halves (row_grp tiling). exp on ScalarE.
  ctx^T_aug [65, q] = v_aug.T @ expS^T  (row 64 = denominator)
  normalize: recip(row 64), gpsimd partition_broadcast, DVE multiply
  out_partial [4096, 1024] = ctx^T.T @ Wo_slice, copied to bf16 and
  summed on host.

Scheduling: projection chains for K/V beyond chunk 0 are woven into the
attention kc-loops as PE filler (region-level Tile deps allow partial
reads of KTt/vaug), and each chunk's output projection is woven into the
next chunk's kc-loop so the single po PSUM bank never stalls the PE.
The normalize chain is kept to few, fp32 engine ops (engine ops carry
~650ns fixed cost, and the woven units gate on counting semaphores =
all earlier DVE/GpSimd work), the last kc's exp is split in half to
give the chunk tail a head start, and the final chunk's tail pipelines
its out-proj matmuls over the freed psS banks with PSUM->bf16 casts
alternating ScalarE/DVE.
The 1/sqrt(D) scale is folded into Wq/bq on the host (exact: 0.125).

Measured (8 axon trn2 cores, nominal 2.4GHz clock): ~222.5us HW exec
(222345/222681), rel err 2.4e-3 (baseline: 250575ns). Note the device clock is bimodal
across runs (~379ns vs ~454ns per 512-col matmul = 2.4 vs 2.0 GHz) and
there is +-2us run noise at fixed clock; compare timings via the median
matmul duration in the NTFF trace.
"""

import numpy as np
import ml_dtypes

import concourse.bass as bass
from concourse import bacc
import concourse.tile as tile
from concourse import mybir, library_config
from concourse.bass_utils import run_bass_kernel_spmd

BF16 = mybir.dt.bfloat16
F32 = mybir.dt.float32

B, N, C = 2, 2048, 1024
H, D = 16, 64
T = B * N              # 4096 tokens
HPC = H // 8           # heads per core = 2
DPC = HPC * D          # head dims per core = 128


def build_core_program(nc):
    """Emit the per-core SPMD program. Same program on all 8 cores;
    per-core data differences come from the input maps."""
    xT = nc.dram_tensor("xT", [C, T], BF16, kind="ExternalInput").ap()
    wq = nc.dram_tensor("wq", [C, DPC], BF16, kind="ExternalInput").ap()
    wk = nc.dram_tensor("wk", [C, DPC], BF16, kind="ExternalInput").ap()
    wv = nc.dram_tensor("wv", [C, DPC], BF16, kind="ExternalInput").ap()
    wo = nc.dram_tensor("wo", [DPC, C], BF16, kind="ExternalInput").ap()
    bqkv = nc.dram_tensor("bqkv", [DPC, 3], F32, kind="ExternalInput").ap()
    iden = nc.dram_tensor("iden", [128, 128], BF16, kind="ExternalInput").ap()
    out = nc.dram_tensor("out", [T, C], BF16, kind="ExternalOutput").ap()

    KCH = C // 128     # 8 contraction chunks for projections
    NCH = T // 512     # 8 token chunks of 512
    KT16 = N // 128    # 16 key tiles per batch

    with tile.TileContext(nc) as tc:
        with tc.tile_pool(name="singles", bufs=1) as singles:
            nc.gpsimd.load_library(library_config.proxy)

            # DMA priority order: iden (warmup), wv weights + x chunk 0
            # (first projection chain), then the rest; wo last (first used
            # ~40us in).
            id_sb = singles.tile([128, 128], BF16, tag="iden")
            nc.sync.dma_start(out=id_sb, in_=iden)

            w_sb = {}
            w_tiles = {}
            for nm, w in (("wv", wv), ("wk", wk), ("wq", wq)):
                t = singles.tile([128, KCH, DPC], BF16, tag=f"w{nm}",
                                 name=f"w{nm}")
                w_tiles[nm] = (t, w)
                w_sb[nm] = [t[:, k, :] for k in range(KCH)]

            def dma_w(nm):
                t, w = w_tiles[nm]
                nc.sync.dma_start(
                    out=t, in_=w.rearrange("(k p) j -> p k j", p=128))

            # per-(k, token-chunk) [128, 512] tiles, DMA'd separately so a
            # projection chain's k-th matmul can start as soon as its own
            # 128KB tile lands (progressive arrival beats fewer/bigger
            # transfers here — the source rows are 1KB either way)
            xt = [[singles.tile([128, 512], BF16, tag=f"xt{k}_{t}",
                                name=f"xt{k}_{t}")
                   for t in range(NCH)] for k in range(KCH)]

            def dma_x(t):
                for k in range(KCH):
                    nc.sync.dma_start(
                        out=xt[k][t],
                        in_=xT[k * 128:(k + 1) * 128, t * 512:(t + 1) * 512])

            bqkv_sb = singles.tile([DPC, 3], F32, tag="bqkv")
            wo_sb = singles.tile([DPC, C], BF16, tag="wo")

            dma_w("wv")
            dma_x(0)
            dma_w("wk")
            dma_x(1)
            dma_w("wq")
            nc.sync.dma_start(out=bqkv_sb, in_=bqkv)
            for t in range(2, NCH):
                dma_x(t)
            nc.sync.dma_start(out=wo_sb, in_=wo)

            b_sb = {"bq": bqkv_sb[:, 0:1], "bk": bqkv_sb[:, 1:2],
                    "bv": bqkv_sb[:, 2:3]}

            # warmup source: memset early so the warmup matmuls (inside the
            # PSUM pools below) don't queue behind the vaug memsets on DVE
            wsrc = singles.tile([128, 128], BF16, tag="wsrc")
            nc.vector.memset(wsrc, 0.5)

            QT = singles.tile([128, T], BF16, tag="QT")
            KTt = singles.tile([128, T], BF16, tag="KT")
            VT = singles.tile([128, T], BF16, tag="VT")
            ctxTn = singles.tile([128, T], BF16, tag="ctxTn")
            vaug = [[singles.tile([128, KT16, D + 1], BF16,
                                  tag=f"vaug{b}{h}", name=f"vaug{b}{h}")
                     for h in range(HPC)] for b in range(B)]
            for b in range(B):
                for h in range(HPC):
                    nc.vector.memset(vaug[b][h], 1.0)

            # One unified PSUM layout for the whole kernel (8 banks:
            # pj 1 + po 1 + s 2x2 + ctx 2).
            with tc.tile_pool(name="psP", bufs=1, space="PSUM") as psP, \
                    tc.tile_pool(name="psO", bufs=1, space="PSUM") as psO, \
                    tc.tile_pool(name="psS", bufs=2, space="PSUM") as psS, \
                    tc.tile_pool(name="psC", bufs=1, space="PSUM") as psC, \
                    tc.tile_pool(name="esb", bufs=6) as esb, \
                    tc.tile_pool(name="nrm", bufs=3) as nrm, \
                    tc.tile_pool(name="csb", bufs=3) as csb, \
                    tc.tile_pool(name="osb", bufs=3) as osb:

                # keep PE busy (p-state ramp) while xT streams in; matmul
                # on a memset tile so warmup needs no DMA (starts ~4.5us
                # in, right after the engine preamble)
                for wu in range(16):
                    ptw = psS.tile([128, 128], F32, tag="s", name="ptw")
                    nc.tensor.matmul(out=ptw, lhsT=wsrc, rhs=wsrc,
                                     start=True, stop=True)
                startup_ctr = [0]

                def emit_proj(nm, dstT, nch, act_bias=False):
                    # Startup chains rotate over 4 banks (po + the ctx pair
                    # are idle pre-attention) with the bias on ScalarE;
                    # mid-attention fill chains use the single pj bank with
                    # the bias on DVE (chains are spaced kc's apart so the
                    # drain hides).
                    if act_bias:
                        slots = [(psO, "po"), (psC, "ctx0"), (psC, "ctx1"),
                                 (psP, "pj")]
                        pool, tg = slots[startup_ctr[0] % len(slots)]
                        startup_ctr[0] += 1
                    else:
                        pool, tg = psP, "pj"
                    ps = pool.tile([128, 512], F32, tag=tg, name="pj")
                    for k in range(KCH):
                        nc.tensor.matmul(
                            out=ps, lhsT=w_sb[nm][k], rhs=xt[k][nch],
                            start=(k == 0), stop=(k == KCH - 1))
                    dst = dstT[:, nch * 512:(nch + 1) * 512]
                    if act_bias:
                        # ScalarE is idle before attention starts
                        nc.scalar.activation(
                            out=dst, in_=ps,
                            func=mybir.ActivationFunctionType.Identity,
                            bias=b_sb["b" + nm[1]], scale=1.0)
                    else:
                        nc.vector.tensor_scalar_add(
                            out=dst, in0=ps, scalar1=b_sb["b" + nm[1]])
                    if nm == "wv":
                        # transpose the 4 just-projected 128-token tiles of V
                        # into v_aug [keys, 65] per head
                        for t16 in range(nch * 4, nch * 4 + 4):
                            b, bt = divmod(t16, KT16)
                            pt = psO.tile([128, 128], BF16, tag="po",
                                          name="pt")
                            base = t16 * 128
                            nc.tensor.transpose(
                                pt, VT[:, base:base + 128], id_sb)
                            nc.vector.tensor_copy(
                                out=vaug[b][0][:, bt, 0:D], in_=pt[:, 0:D])
                            nc.vector.tensor_copy(
                                out=vaug[b][1][:, bt, 0:D], in_=pt[:, D:2 * D])

                def emit_attention(b, qch, fillers, last=False,
                                   alt_banks=False, units_in_psC=False):
                    q0 = b * N + qch * 512
                    if alt_banks:
                        # chunks 5-7 have no projection chains left, so the
                        # pj/po banks are free: park the ctx accumulators
                        # there and let this chunk's woven out-proj units
                        # use psC — decouples the chunk boundary from the
                        # previous normalize's PSUM reads.
                        ctx = [psP.tile([D + 1, 512], F32, tag="pj",
                                        name="ctx0"),
                               psO.tile([D + 1, 512], F32, tag="po",
                                        name="ctx1")]
                    else:
                        ctx = [psC.tile([D + 1, 512], F32, tag=f"ctx{h}",
                                        name=f"ctx{h}")
                               for h in range(HPC)]

                    # 2-kc groups: [S,S][exp,exp][fills][C,C]. Entering or
                    # leaving a row-tiled scores pair costs ~+100ns on the
                    # PE (the full-array successor waits for both subarray
                    # pipelines); grouping halves those crossings and the
                    # fills sit on the S->C boundary that was paying the
                    # penalty anyway.
                    for kc0 in range(0, KT16, 2):
                        pSs = []
                        for kc in (kc0, kc0 + 1):
                            k0 = b * N + kc * 128
                            pS = psS.tile([128, 1024], F32, tag="s",
                                          name="s")
                            pSs.append(pS)
                            for h in range(HPC):
                                nc.tensor.matmul(
                                    out=pS[:, h * 512:(h + 1) * 512],
                                    lhsT=KTt[h * D:(h + 1) * D,
                                             k0:k0 + 128],
                                    rhs=QT[h * D:(h + 1) * D,
                                           q0:q0 + 512],
                                    start=True, stop=True)
                        eSs = []
                        for kc, pS in zip((kc0, kc0 + 1), pSs):
                            eS = esb.tile([128, 1024], BF16, tag="e",
                                          name="e")
                            eSs.append(eS)
                            if kc == KT16 - 1:
                                # split the last exp so ctx(15) and the
                                # normalize get a ~0.5us head start
                                for h in range(HPC):
                                    nc.scalar.activation(
                                        eS[:, h * 512:(h + 1) * 512],
                                        pS[:, h * 512:(h + 1) * 512],
                                        mybir.ActivationFunctionType.Exp)
                            else:
                                nc.scalar.activation(
                                    eS, pS,
                                    mybir.ActivationFunctionType.Exp)
                        for kc in (kc0, kc0 + 1):
                            for f in fillers.get(kc, ()):
                                f()
                        for kc, eS in zip((kc0, kc0 + 1), eSs):
                            for h in range(HPC):
                                nc.tensor.matmul(
                                    out=ctx[h],
                                    lhsT=vaug[b][h][:, kc, :],
                                    rhs=eS[:, h * 512:(h + 1) * 512],
                                    start=(kc == 0),
                                    stop=(kc == KT16 - 1))
                    # normalize: rows 0..63 / row 64, into stacked ctxTn.
                    # The woven out-proj units wait on COUNTING semaphores —
                    # all earlier DVE/GpSimd work must finish — so minimize
                    # the op count: ONE [65,512] f32 copy per head (frees
                    # the PSUM bank and feeds both the reciprocal and the
                    # multiplies), reciprocal off its SBUF row 64, f32
                    # broadcast (same ~1us as bf16, no cast needed). In the
                    # last chunk the copies go to ScalarE (idle once the
                    # exps are done) to shorten the DVE chain further.
                    rcs = []
                    ctxss = []
                    for h in range(HPC):
                        cp = csb.tile([D + 1, 512], F32, tag=f"ctxs{h}",
                                      name=f"ctxs{h}")
                        if last:
                            nc.scalar.copy(cp, ctx[h])
                        else:
                            nc.vector.tensor_copy(cp, ctx[h])
                        # the custom-DVE reciprocal needs a partition-0 fp32
                        # SBUF input — a base-64 slice (or PSUM) gives
                        # garbage, so extract the denominator row first
                        dn = nrm.tile([1, 512], F32, tag=f"dn{h}",
                                      name=f"dn{h}")
                        nc.vector.tensor_copy(dn, ctx[h][D:D + 1, :])
                        rc = nrm.tile([1, 512], F32, tag=f"rc{h}",
                                      name=f"rc{h}")
                        nc.vector.reciprocal_approx_fast(rc, dn)
                        ctxss.append(cp)
                        rcs.append(rc)
                    bcs = []
                    for h in range(HPC):
                        bc = nrm.tile([D, 512], F32, tag=f"bc{h}",
                                      name=f"bc{h}")
                        nc.gpsimd.partition_broadcast(bc, rcs[h])
                        bcs.append(bc)

                    def emit_mul(h, lo, hi):
                        # h0's multiply runs on GpSimd (idle engine, no
                        # partition shift); h1's needs the base-64 write,
                        # keep it on DVE where that is proven.
                        eng = nc.gpsimd if h == 0 else nc.vector
                        eng.tensor_mul(
                            out=ctxTn[h * D:(h + 1) * D, q0 + lo:q0 + hi],
                            in0=ctxss[h][0:D, lo:hi], in1=bcs[h][:, lo:hi])

                    def emit_op(t4, nch2, po, ot_slice):
                        tok = q0 + t4 * 128
                        nc.tensor.matmul(
                            out=po,
                            lhsT=ctxTn[:, tok:tok + 128],
                            rhs=wo_sb[:, nch2 * 512:(nch2 + 1) * 512],
                            start=True, stop=True)
                        nc.vector.tensor_copy(ot_slice, po)

                    if not last:
                        for h in range(HPC):
                            emit_mul(h, 0, 512)

                        # out-projection as 8 weavable units; each does one
                        # [128-token x 512-col] matmul + bf16 copy, DMA per
                        # 4-tile group. Woven into the NEXT chunk's kc loop
                        # so the po bank drain hides under attention.
                        ot_box = [None]

                        def op_unit(j):
                            nch2, t4 = divmod(j, 4)

                            def run():
                                if t4 == 0:
                                    ot_box[0] = osb.tile(
                                        [128, 4, 512], BF16, tag="ot",
                                        name="ot")
                                if units_in_psC:
                                    po = psC.tile([128, 512], F32,
                                                  tag=f"ctx{j % 2}",
                                                  name="po")
                                else:
                                    po = psO.tile([128, 512], F32,
                                                  tag="po", name="po")
                                emit_op(t4, nch2, po, ot_box[0][:, t4, :])
                                if t4 == 3:
                                    nc.sync.dma_start(
                                        out=out[q0:q0 + 512,
                                                nch2 * 512:(nch2 + 1) * 512
                                                ].rearrange(
                                                    "(t p) c -> p t c", p=128),
                                        in_=ot_box[0])
                            return run

                        return [op_unit(j) for j in range(8)]

                    # final chunk: per-qtile pipeline, matmuls on the four
                    # freed psS banks, PSUM->bf16 casts alternating between
                    # ScalarE (idle after the last exp) and DVE so they
                    # drain in parallel with the matmul stream
                    for t4 in range(4):
                        for h in range(HPC):
                            emit_mul(h, t4 * 128, (t4 + 1) * 128)
                    for t4 in range(4):
                        for nch2 in range(2):
                            tok = q0 + t4 * 128
                            po = psS.tile([128, 512], F32, tag="s",
                                          name="po")
                            ot = osb.tile([128, 512], BF16, tag="otl",
                                          name="otl", bufs=4)
                            nc.tensor.matmul(
                                out=po,
                                lhsT=ctxTn[:, tok:tok + 128],
                                rhs=wo_sb[:, nch2 * 512:(nch2 + 1) * 512],
                                start=True, stop=True)
                            if nch2 == 0:
                                nc.scalar.copy(ot, po)
                            else:
                                nc.vector.tensor_copy(ot, po)
                            nc.sync.dma_start(
                                out=out[tok:tok + 128,
                                        nch2 * 512:(nch2 + 1) * 512],
                                in_=ot)
                    return None

                # Startup: only chunk-0 projections of V/K/Q, then attention
                # begins; the rest of b0's K/V chains weave into chunk 0's
                # kc loop (scores kc needs K chunk kc//4, ctx kc needs V
                # chunk kc//4, each emitted >=2 kc ahead).
                emit_proj("wv", VT, 0, act_bias=True)
                emit_proj("wk", KTt, 0, act_bias=True)
                emit_proj("wq", QT, 0, act_bias=True)

                # proj fill chains per chunk index i (woven at odd kc slots;
                # out-proj units go at even slots). All Q projections are
                # front-loaded into chunks 0-4 so chunks 5-7 leave pj/po
                # free for their ctx accumulators (alt_banks).
                proj_fills = {
                    0: {1: [("wk", KTt, 1)], 3: [("wv", VT, 1)],
                        5: [("wk", KTt, 2)], 7: [("wv", VT, 2)],
                        9: [("wk", KTt, 3)], 11: [("wv", VT, 3)]},
                    1: {5: [("wk", KTt, 4)], 7: [("wv", VT, 4)]},
                    2: {5: [("wk", KTt, 5)], 7: [("wv", VT, 5)]},
                    3: {5: [("wk", KTt, 6)], 7: [("wv", VT, 6)]},
                    4: {1: [("wk", KTt, 7)], 3: [("wv", VT, 7)]},
                }
                pending_ops = None
                for i in range(8):
                    fillers = {}
                    for kc, chains in proj_fills.get(i, {}).items():
                        fillers[kc] = [
                            (lambda a: lambda: emit_proj(*a))(a)
                            for a in chains]
                    if pending_ops is not None:
                        # weave previous chunk's out-proj into the kc loop.
                        # Unit 0 reads ctxTn, ready only ~4-5us after the
                        # boundary; chunks with proj fill chains (i<=4)
                        # have enough other PE work for early slots, the
                        # later chunks need the weave pushed to slot 6+.
                        if i <= 4:
                            slots = [2, 4, 6, 8, 10, 12, 14, 15]
                        else:
                            slots = [6, 7, 8, 9, 10, 11, 12, 13]
                        for j, u in enumerate(pending_ops):
                            fillers.setdefault(slots[j], []).append(u)
                    if i + 1 < 8:
                        # slot 9 (not 13): the Q chain's DVE bias must
                        # drain before the boundary normalize chain, which
                        # gates the next chunk's ctx banks via counting
                        # semaphores. Chunk 0's slots 9/11 hold wk3/wv3,
                        # so its Q goes at 13 (its boundary is clean
                        # anyway — no unit weave in chunk 1's early slots).
                        qslot = 13 if i == 0 else 9
                        fillers.setdefault(qslot, []).append(
                            (lambda n: lambda: emit_proj("wq", QT, n))(i + 1))
                    pending_ops = emit_attention(
                        i // 4, i % 4, fillers, last=(i == 7))
    return nc


_NC_CACHE = None


def _get_nc():
    global _NC_CACHE
    if _NC_CACHE is None:
        nc = bacc.Bacc("TRN2", target_bir_lowering=False)
        build_core_program(nc)
        nc.finalize()
        _NC_CACHE = nc
    return _NC_CACHE


def make_in_maps(x, Wq, bq, Wk, bk, Wv, bv, Wo):
    bf = ml_dtypes.bfloat16
    x = np.asarray(x, np.float32).reshape(T, C)
    xT_bf = np.ascontiguousarray(x.T).astype(bf)
    iden = np.eye(128, dtype=bf)
    Wq = np.asarray(Wq, np.float32)
    Wk = np.asarray(Wk, np.float32)
    Wv = np.asarray(Wv, np.float32)
    Wo = np.asarray(Wo, np.float32)
    bq = np.asarray(bq, np.float32)
    bk = np.asarray(bk, np.float32)
    bv = np.asarray(bv, np.float32)
    in_maps = []
    for cidx in range(8):
        hs = slice(cidx * DPC, (cidx + 1) * DPC)
        in_maps.append(dict(
            xT=xT_bf,
            wq=np.ascontiguousarray(Wq[:, hs] * 0.125).astype(bf),
            wk=np.ascontiguousarray(Wk[:, hs]).astype(bf),
            wv=np.ascontiguousarray(Wv[:, hs]).astype(bf),
            wo=np.ascontiguousarray(Wo[hs, :]).astype(bf),
            bqkv=np.stack([bq[hs] * 0.125, bk[hs], bv[hs]],
                          axis=1).astype(np.float32),
            iden=iden,
        ))
    return in_maps


def kernel(x, Wq, bq, Wk, bk, Wv, bv, Wo, bo, _trace=False, _trace_kwargs=None):
    in_maps = make_in_maps(x, Wq, bq, Wk, bk, Wv, bv, Wo)
    nc = _get_nc()
    res = run_bass_kernel_spmd(
        nc, in_maps, core_ids=list(range(8)),
        trace=_trace, **(_trace_kwargs or {}))
    acc = res.results[0]["out"].astype(np.float32)
    for cidx in range(1, 8):
        acc += res.results[cidx]["out"]
    acc += np.asarray(bo, np.float32)[None, :]
    out = acc.reshape(B, N, C)
    kernel.last_results = res
    return out


# revision 69
# speedup vs baseline: 1.0127x; 1.0019x over previous
"""Multi-head attention (B=2, N=2048, C=1024, H=16, D=64) on 8 TRN2 cores.

Sharding: tensor-parallel over heads — 2 heads per core. Each core computes
Q/K/V projections for its 2 heads, attention, and a partial output
projection (its heads' slice of Wo). Host sums the 8 partial outputs + bo.

Per-core dataflow (all matmul inputs bf16, PSUM accumulation fp32):
  xT [1024, 4096] (x transposed on host, replicated to all cores),
  loaded token-major as 8x8 tiles [128, 512] so the first projection
  chain can start after ~1MB instead of 4MB.
  QT/KT = W.T @ x.T   -> [128 (2 heads x 64), 4096]  (lhsT=W chunk, rhs=xT)
  VT likewise, then PE-transposed into v_aug [keys, 65] per head
  (65th column = ones -> softmax denominator comes out of the ctx matmul)
  S^T = K @ Q.T  -> [keys, q] in PSUM; the two heads' matmuls run
  concurrently on the PE row-halves (row_grp tiling). exp on ScalarE.
  ctx^T_aug [65, q] = v_aug.T @ expS^T  (row 64 = denominator)
  normalize: recip(row 64), gpsimd partition_broadcast, DVE multiply
  out_partial [4096, 1024] = ctx^T.T @ Wo_slice, copied to bf16 and
  summed on host.

Scheduling: projection chains for K/V beyond chunk 0 are woven into the
attention kc-loops as PE filler (region-level Tile deps allow partial
reads of KTt/vaug), and each chunk's output projection is woven into the
next chunk's kc-loop so the single po PSUM bank never stalls the PE.
The normalize chain is kept to few, fp32 engine ops (engine ops carry
~650ns fixed cost, and the woven units gate on counting semaphores =
all earlier DVE/GpSimd work), the last kc's exp is split in half to
give the chunk tail a head start, and the final chunk's tail pipelines
its out-proj matmuls over the freed psS banks with PSUM->bf16 casts
alternating ScalarE/DVE.
The 1/sqrt(D) scale is folded into Wq/bq on the host (exact: 0.125).

Measured (8 axon trn2 cores, nominal 2.4GHz clock): ~222.1us HW exec
(222345/222681/221318), rel err 2.4e-3 (baseline: 250575ns). Note the device clock is bimodal
across runs (~379ns vs ~454ns per 512-col matmul = 2.4 vs 2.0 GHz) and
there is +-2us run noise at fixed clock; compare timings via the median
matmul duration in the NTFF trace.
"""

import numpy as np
import ml_dtypes

import concourse.bass as bass
from concourse import bacc
import concourse.tile as tile
from concourse import mybir, library_config
from concourse.bass_utils import run_bass_kernel_spmd

BF16 = mybir.dt.bfloat16
F32 = mybir.dt.float32

B, N, C = 2, 2048, 1024
H, D = 16, 64
T = B * N              # 4096 tokens
HPC = H // 8           # heads per core = 2
DPC = HPC * D          # head dims per core = 128


def build_core_program(nc):
    """Emit the per-core SPMD program. Same program on all 8 cores;
    per-core data differences come from the input maps."""
    xT = nc.dram_tensor("xT", [C, T], BF16, kind="ExternalInput").ap()
    wq = nc.dram_tensor("wq", [C, DPC], BF16, kind="ExternalInput").ap()
    wk = nc.dram_tensor("wk", [C, DPC], BF16, kind="ExternalInput").ap()
    wv = nc.dram_tensor("wv", [C, DPC], BF16, kind="ExternalInput").ap()
    wo = nc.dram_tensor("wo", [DPC, C], BF16, kind="ExternalInput").ap()
    bqkv = nc.dram_tensor("bqkv", [DPC, 3], F32, kind="ExternalInput").ap()
    iden = nc.dram_tensor("iden", [128, 128], BF16, kind="ExternalInput").ap()
    out = nc.dram_tensor("out", [T, C], BF16, kind="ExternalOutput").ap()

    KCH = C // 128     # 8 contraction chunks for projections
    NCH = T // 512     # 8 token chunks of 512
    KT16 = N // 128    # 16 key tiles per batch

    with tile.TileContext(nc) as tc:
        with tc.tile_pool(name="singles", bufs=1) as singles:
            nc.gpsimd.load_library(library_config.proxy)

            # DMA priority order: iden (warmup), wv weights + x chunk 0
            # (first projection chain), then the rest; wo last (first used
            # ~40us in).
            id_sb = singles.tile([128, 128], BF16, tag="iden")
            nc.sync.dma_start(out=id_sb, in_=iden)

            w_sb = {}
            w_tiles = {}
            for nm, w in (("wv", wv), ("wk", wk), ("wq", wq)):
                t = singles.tile([128, KCH, DPC], BF16, tag=f"w{nm}",
                                 name=f"w{nm}")
                w_tiles[nm] = (t, w)
                w_sb[nm] = [t[:, k, :] for k in range(KCH)]

            def dma_w(nm):
                t, w = w_tiles[nm]
                nc.sync.dma_start(
                    out=t, in_=w.rearrange("(k p) j -> p k j", p=128))

            # per-(k, token-chunk) [128, 512] tiles, DMA'd separately so a
            # projection chain's k-th matmul can start as soon as its own
            # 128KB tile lands (progressive arrival beats fewer/bigger
            # transfers here — the source rows are 1KB either way)
            xt = [[singles.tile([128, 512], BF16, tag=f"xt{k}_{t}",
                                name=f"xt{k}_{t}")
                   for t in range(NCH)] for k in range(KCH)]

            def dma_x(t):
                for k in range(KCH):
                    nc.sync.dma_start(
                        out=xt[k][t],
                        in_=xT[k * 128:(k + 1) * 128, t * 512:(t + 1) * 512])

            bqkv_sb = singles.tile([DPC, 3], F32, tag="bqkv")
            wo_sb = singles.tile([DPC, C], BF16, tag="wo")

            dma_w("wv")
            dma_x(0)
            dma_w("wk")
            dma_x(1)
            dma_w("wq")
            nc.sync.dma_start(out=bqkv_sb, in_=bqkv)
            for t in range(2, NCH):
                dma_x(t)
            nc.sync.dma_start(out=wo_sb, in_=wo)

            b_sb = {"bq": bqkv_sb[:, 0:1], "bk": bqkv_sb[:, 1:2],
                    "bv": bqkv_sb[:, 2:3]}

            # warmup source: memset early so the warmup matmuls (inside the
            # PSUM pools below) don't queue behind the vaug memsets on DVE
            wsrc = singles.tile([128, 128], BF16, tag="wsrc")
            nc.vector.memset(wsrc, 0.5)

            QT = singles.tile([128, T], BF16, tag="QT")
            KTt = singles.tile([128, T], BF16, tag="KT")
            VT = singles.tile([128, T], BF16, tag="VT")
            ctxTn = singles.tile([128, T], BF16, tag="ctxTn")
            vaug = [[singles.tile([128, KT16, D + 1], BF16,
                                  tag=f"vaug{b}{h}", name=f"vaug{b}{h}")
                     for h in range(HPC)] for b in range(B)]
            for b in range(B):
                for h in range(HPC):
                    nc.vector.memset(vaug[b][h], 1.0)

            # One unified PSUM layout for the whole kernel (8 banks:
            # pj 1 + po 1 + s 2x2 + ctx 2).
            with tc.tile_pool(name="psP", bufs=1, space="PSUM") as psP, \
                    tc.tile_pool(name="psO", bufs=1, space="PSUM") as psO, \
                    tc.tile_pool(name="psS", bufs=2, space="PSUM") as psS, \
                    tc.tile_pool(name="psC", bufs=1, space="PSUM") as psC, \
                    tc.tile_pool(name="esb", bufs=6) as esb, \
                    tc.tile_pool(name="nrm", bufs=3) as nrm, \
                    tc.tile_pool(name="csb", bufs=3) as csb, \
                    tc.tile_pool(name="osb", bufs=3) as osb:

                # keep PE busy (p-state ramp) while xT streams in; matmul
                # on a memset tile so warmup needs no DMA (starts ~4.5us
                # in, right after the engine preamble)
                for wu in range(16):
                    ptw = psS.tile([128, 128], F32, tag="s", name="ptw")
                    nc.tensor.matmul(out=ptw, lhsT=wsrc, rhs=wsrc,
                                     start=True, stop=True)
                startup_ctr = [0]

                def emit_proj(nm, dstT, nch, act_bias=False):
                    # Startup chains rotate over 4 banks (po + the ctx pair
                    # are idle pre-attention) with the bias on ScalarE;
                    # mid-attention fill chains use the single pj bank with
                    # the bias on DVE (chains are spaced kc's apart so the
                    # drain hides).
                    if act_bias:
                        slots = [(psO, "po"), (psC, "ctx0"), (psC, "ctx1"),
                                 (psP, "pj")]
                        pool, tg = slots[startup_ctr[0] % len(slots)]
                        startup_ctr[0] += 1
                    else:
                        pool, tg = psP, "pj"
                    ps = pool.tile([128, 512], F32, tag=tg, name="pj")
                    for k in range(KCH):
                        nc.tensor.matmul(
                            out=ps, lhsT=w_sb[nm][k], rhs=xt[k][nch],
                            start=(k == 0), stop=(k == KCH - 1))
                    dst = dstT[:, nch * 512:(nch + 1) * 512]
                    if act_bias:
                        # ScalarE is idle before attention starts
                        nc.scalar.activation(
                            out=dst, in_=ps,
                            func=mybir.ActivationFunctionType.Identity,
                            bias=b_sb["b" + nm[1]], scale=1.0)
                    else:
                        nc.vector.tensor_scalar_add(
                            out=dst, in0=ps, scalar1=b_sb["b" + nm[1]])
                    if nm == "wv":
                        # transpose the 4 just-projected 128-token tiles of V
                        # into v_aug [keys, 65] per head
                        for t16 in range(nch * 4, nch * 4 + 4):
                            b, bt = divmod(t16, KT16)
                            pt = psO.tile([128, 128], BF16, tag="po",
                                          name="pt")
                            base = t16 * 128
                            nc.tensor.transpose(
                                pt, VT[:, base:base + 128], id_sb)
                            nc.vector.tensor_copy(
                                out=vaug[b][0][:, bt, 0:D], in_=pt[:, 0:D])
                            nc.vector.tensor_copy(
                                out=vaug[b][1][:, bt, 0:D], in_=pt[:, D:2 * D])

                def emit_attention(b, qch, fillers, last=False,
                                   alt_banks=False, units_in_psC=False):
                    q0 = b * N + qch * 512
                    if alt_banks:
                        # chunks 5-7 have no projection chains left, so the
                        # pj/po banks are free: park the ctx accumulators
                        # there and let this chunk's woven out-proj units
                        # use psC — decouples the chunk boundary from the
                        # previous normalize's PSUM reads.
                        ctx = [psP.tile([D + 1, 512], F32, tag="pj",
                                        name="ctx0"),
                               psO.tile([D + 1, 512], F32, tag="po",
                                        name="ctx1")]
                    else:
                        ctx = [psC.tile([D + 1, 512], F32, tag=f"ctx{h}",
                                        name=f"ctx{h}")
                               for h in range(HPC)]

                    # 2-kc groups: [S,S][exp,exp][fills][C,C]. Entering or
                    # leaving a row-tiled scores pair costs ~+100ns on the
                    # PE (the full-array successor waits for both subarray
                    # pipelines); grouping halves those crossings and the
                    # fills sit on the S->C boundary that was paying the
                    # penalty anyway.
                    for kc0 in range(0, KT16, 2):
                        pSs = []
                        for kc in (kc0, kc0 + 1):
                            k0 = b * N + kc * 128
                            pS = psS.tile([128, 1024], F32, tag="s",
                                          name="s")
                            pSs.append(pS)
                            for h in range(HPC):
                                nc.tensor.matmul(
                                    out=pS[:, h * 512:(h + 1) * 512],
                                    lhsT=KTt[h * D:(h + 1) * D,
                                             k0:k0 + 128],
                                    rhs=QT[h * D:(h + 1) * D,
                                           q0:q0 + 512],
                                    start=True, stop=True)
                        eSs = []
                        for kc, pS in zip((kc0, kc0 + 1), pSs):
                            eS = esb.tile([128, 1024], BF16, tag="e",
                                          name="e")
                            eSs.append(eS)
                            if kc == KT16 - 1:
                                # split the last exp so ctx(15) and the
                                # normalize get a ~0.5us head start
                                for h in range(HPC):
                                    nc.scalar.activation(
                                        eS[:, h * 512:(h + 1) * 512],
                                        pS[:, h * 512:(h + 1) * 512],
                                        mybir.ActivationFunctionType.Exp)
                            else:
                                nc.scalar.activation(
                                    eS, pS,
                                    mybir.ActivationFunctionType.Exp)
                        for kc in (kc0, kc0 + 1):
                            for f in fillers.get(kc, ()):
                                f()
                        for kc, eS in zip((kc0, kc0 + 1), eSs):
                            for h in range(HPC):
                                nc.tensor.matmul(
                                    out=ctx[h],
                                    lhsT=vaug[b][h][:, kc, :],
                                    rhs=eS[:, h * 512:(h + 1) * 512],
                                    start=(kc == 0),
                                    stop=(kc == KT16 - 1))
                    # normalize: rows 0..63 / row 64, into stacked ctxTn.
                    # The woven out-proj units wait on COUNTING semaphores —
                    # all earlier DVE/GpSimd work must finish — so minimize
                    # the op count: ONE [65,512] f32 copy per head (frees
                    # the PSUM bank and feeds both the reciprocal and the
                    # multiplies), reciprocal off its SBUF row 64, f32
                    # broadcast (same ~1us as bf16, no cast needed). In the
                    # last chunk the copies go to ScalarE (idle once the
                    # exps are done) to shorten the DVE chain further.
                    rcs = []
                    ctxss = []
                    for h in range(HPC):
                        cp = csb.tile([D + 1, 512], F32, tag=f"ctxs{h}",
                                      name=f"ctxs{h}")
                        if last:
                            nc.scalar.copy(cp, ctx[h])
                        else:
                            nc.vector.tensor_copy(cp, ctx[h])
                        # the custom-DVE reciprocal needs a partition-0 fp32
                        # SBUF input — a base-64 slice (or PSUM) gives
                        # garbage, so extract the denominator row first
                        dn = nrm.tile([1, 512], F32, tag=f"dn{h}",
                                      name=f"dn{h}")
                        nc.vector.tensor_copy(dn, ctx[h][D:D + 1, :])
                        rc = nrm.tile([1, 512], F32, tag=f"rc{h}",
                                      name=f"rc{h}")
                        nc.vector.reciprocal_approx_fast(rc, dn)
                        ctxss.append(cp)
                        rcs.append(rc)
                    bcs = []
                    for h in range(HPC):
                        bc = nrm.tile([D, 512], F32, tag=f"bc{h}",
                                      name=f"bc{h}")
                        nc.gpsimd.partition_broadcast(bc, rcs[h])
                        bcs.append(bc)

                    def emit_mul(h, lo, hi):
                        # h0's multiply runs on GpSimd (idle engine, no
                        # partition shift); h1's needs the base-64 write,
                        # keep it on DVE where that is proven.
                        eng = nc.gpsimd if h == 0 else nc.vector
                        eng.tensor_mul(
                            out=ctxTn[h * D:(h + 1) * D, q0 + lo:q0 + hi],
                            in0=ctxss[h][0:D, lo:hi], in1=bcs[h][:, lo:hi])

                    def emit_op(t4, nch2, po, ot_slice):
                        tok = q0 + t4 * 128
                        nc.tensor.matmul(
                            out=po,
                            lhsT=ctxTn[:, tok:tok + 128],
                            rhs=wo_sb[:, nch2 * 512:(nch2 + 1) * 512],
                            start=True, stop=True)
                        nc.vector.tensor_copy(ot_slice, po)

                    if not last:
                        for h in range(HPC):
                            emit_mul(h, 0, 512)

                        # out-projection as 8 weavable units; each does one
                        # [128-token x 512-col] matmul + bf16 copy, DMA per
                        # 4-tile group. Woven into the NEXT chunk's kc loop
                        # so the po bank drain hides under attention.
                        ot_box = [None]

                        def op_unit(j):
                            nch2, t4 = divmod(j, 4)

                            def run():
                                if t4 == 0:
                                    ot_box[0] = osb.tile(
                                        [128, 4, 512], BF16, tag="ot",
                                        name="ot")
                                if units_in_psC:
                                    po = psC.tile([128, 512], F32,
                                                  tag=f"ctx{j % 2}",
                                                  name="po")
                                else:
                                    po = psO.tile([128, 512], F32,
                                                  tag="po", name="po")
                                emit_op(t4, nch2, po, ot_box[0][:, t4, :])
                                if t4 == 3:
                                    nc.sync.dma_start(
                                        out=out[q0:q0 + 512,
                                                nch2 * 512:(nch2 + 1) * 512
                                                ].rearrange(
                                                    "(t p) c -> p t c", p=128),
                                        in_=ot_box[0])
                            return run

                        return [op_unit(j) for j in range(8)]

                    # final chunk: per-qtile pipeline, matmuls on the four
                    # freed psS banks, PSUM->bf16 casts alternating between
                    # ScalarE (idle after the last exp) and DVE so they
                    # drain in parallel with the matmul stream
                    for t4 in range(4):
                        for h in range(HPC):
                            emit_mul(h, t4 * 128, (t4 + 1) * 128)
                    for t4 in range(4):
                        for nch2 in range(2):
                            tok = q0 + t4 * 128
                            po = psS.tile([128, 512], F32, tag="s",
                                          name="po")
                            ot = osb.tile([128, 512], BF16, tag="otl",
                                          name="otl", bufs=4)
                            nc.tensor.matmul(
                                out=po,
                                lhsT=ctxTn[:, tok:tok + 128],
                                rhs=wo_sb[:, nch2 * 512:(nch2 + 1) * 512],
                                start=True, stop=True)
                            if nch2 == 0:
                                nc.scalar.copy(ot, po)
                            else:
                                nc.vector.tensor_copy(ot, po)
                            nc.sync.dma_start(
                                out=out[tok:tok + 128,
                                        nch2 * 512:(nch2 + 1) * 512],
                                in_=ot)
                    return None

                # Startup: only chunk-0 projections of V/K/Q, then attention
                # begins; the rest of b0's K/V chains weave into chunk 0's
                # kc loop (scores kc needs K chunk kc//4, ctx kc needs V
                # chunk kc//4, each emitted >=2 kc ahead).
                emit_proj("wv", VT, 0, act_bias=True)
                emit_proj("wk", KTt, 0, act_bias=True)
                emit_proj("wq", QT, 0, act_bias=True)

                # proj fill chains per chunk index i (woven at odd kc slots;
                # out-proj units go at even slots). All Q projections are
                # front-loaded into chunks 0-4 so chunks 5-7 leave pj/po
                # free for their ctx accumulators (alt_banks).
                proj_fills = {
                    0: {1: [("wk", KTt, 1)], 3: [("wv", VT, 1)],
                        5: [("wk", KTt, 2)], 7: [("wv", VT, 2)],
                        9: [("wk", KTt, 3)], 11: [("wv", VT, 3)]},
                    1: {5: [("wk", KTt, 4)], 7: [("wv", VT, 4)]},
                    2: {5: [("wk", KTt, 5)], 7: [("wv", VT, 5)]},
                    3: {5: [("wk", KTt, 6)], 7: [("wv", VT, 6)]},
                    4: {1: [("wk", KTt, 7)], 3: [("wv", VT, 7)]},
                }
                pending_ops = None
                for i in range(8):
                    fillers = {}
                    for kc, chains in proj_fills.get(i, {}).items():
                        fillers[kc] = [
                            (lambda a: lambda: emit_proj(*a))(a)
                            for a in chains]
                    if pending_ops is not None:
                        # weave previous chunk's out-proj into the kc loop.
                        # Unit 0 reads ctxTn, ready only ~4-5us after the
                        # boundary; chunks with proj fill chains (i<=4)
                        # have enough other PE work for early slots, the
                        # later chunks need the weave pushed to slot 6+.
                        if i <= 4:
                            slots = [2, 4, 6, 8, 10, 12, 14, 15]
                        else:
                            slots = [6, 7, 8, 9, 10, 11, 12, 13]
                        for j, u in enumerate(pending_ops):
                            fillers.setdefault(slots[j], []).append(u)
                    if i + 1 < 8:
                        # slot 9 (not 13): the Q chain's DVE bias must
                        # drain before the boundary normalize chain, which
                        # gates the next chunk's ctx banks via counting
                        # semaphores. Chunk 0's slots 9/11 hold wk3/wv3,
                        # so its Q goes at 13 (its boundary is clean
                        # anyway — no unit weave in chunk 1's early slots).
                        qslot = 13 if i == 0 else 9
                        fillers.setdefault(qslot, []).append(
                            (lambda n: lambda: emit_proj("wq", QT, n))(i + 1))
                    pending_ops = emit_attention(
                        i // 4, i % 4, fillers, last=(i == 7))
    return nc


_NC_CACHE = None


def _get_nc():
    global _NC_CACHE
    if _NC_CACHE is None:
        nc = bacc.Bacc("TRN2", target_bir_lowering=False)
        build_core_program(nc)
        nc.finalize()
        _NC_CACHE = nc
    return _NC_CACHE


def make_in_maps(x, Wq, bq, Wk, bk, Wv, bv, Wo):
    bf = ml_dtypes.bfloat16
    x = np.asarray(x, np.float32).reshape(T, C)
    xT_bf = np.ascontiguousarray(x.T).astype(bf)
    iden = np.eye(128, dtype=bf)
    Wq = np.asarray(Wq, np.float32)
    Wk = np.asarray(Wk, np.float32)
    Wv = np.asarray(Wv, np.float32)
    Wo = np.asarray(Wo, np.float32)
    bq = np.asarray(bq, np.float32)
    bk = np.asarray(bk, np.float32)
    bv = np.asarray(bv, np.float32)
    in_maps = []
    for cidx in range(8):
        hs = slice(cidx * DPC, (cidx + 1) * DPC)
        in_maps.append(dict(
            xT=xT_bf,
            wq=np.ascontiguousarray(Wq[:, hs] * 0.125).astype(bf),
            wk=np.ascontiguousarray(Wk[:, hs]).astype(bf),
            wv=np.ascontiguousarray(Wv[:, hs]).astype(bf),
            wo=np.ascontiguousarray(Wo[hs, :]).astype(bf),
            bqkv=np.stack([bq[hs] * 0.125, bk[hs], bv[hs]],
                          axis=1).astype(np.float32),
            iden=iden,
        ))
    return in_maps


def kernel(x, Wq, bq, Wk, bk, Wv, bv, Wo, bo, _trace=False, _trace_kwargs=None):
    in_maps = make_in_maps(x, Wq, bq, Wk, bk, Wv, bv, Wo)
    nc = _get_nc()
    res = run_bass_kernel_spmd(
        nc, in_maps, core_ids=list(range(8)),
        trace=_trace, **(_trace_kwargs or {}))
    acc = res.results[0]["out"].astype(np.float32)
    for cidx in range(1, 8):
        acc += res.results[cidx]["out"]
    acc += np.asarray(bo, np.float32)[None, :]
    out = acc.reshape(B, N, C)
    kernel.last_results = res
    return out
